# revision 27
# baseline (speedup 1.0000x reference)
"""Trainium2 Bass kernel for CrossModalAttention (attention + residual + LayerNorm).

Math: the reference concatenates [query, key_value], projects Q/K/V, attends with an
additive -10000 mask on key columns < N_q, and keeps only the query-row outputs.
exp(-10000 + s) == 0 in fp32, so this is exactly standard cross-attention:
    Q = query @ Wq + bq ; K = key_value @ Wk (+bk) ; V = key_value @ Wv + bv
    C = softmax(Q K^T / 8) V ;  out = LayerNorm(query + C) * gamma + beta
bk shifts every score of a query row equally (score += Q_q . bk for all keys),
so softmax cancels it -> bk is dropped exactly. bv is constant over keys, so
ctx = PV/denom + bv -> folded into the residual (xqb = query + bv) host-side.

Sharding: 8 cores = 2 batches x 4 query-blocks of 512 rows. Each core computes
K/V projections for its batch (duplicated across the 4 cores of a batch) and
full 8-head attention + LN for its query block.

Per-core pipeline (v2):
  - Q/K projections in bf16 (accuracy: fp8 upstream put max-rel-err at the
    2e-2 gate); outputs stored e4m3 for the fp8 DoubleRow scores matmul.
  - V projection in fp8 DR (error averages out through the softmax weights).
  - Wq/Wk columns permuted host-side so each head's 64 hd dims land as two
    32-row half-tiles on partitions 32*(h%4)..+32 -> scores (contraction 64)
    run DoubleRow with zero reshuffling.
  - probs are e5m2 everywhere (e4m3 max ~240 overflows to inf on observed
    scores): ScalarE ACT exp -> e5m2; VectorE Schraudolph affine -> uint8
    bitcast (saturates negative results to +0.0; no NaN cliffs).
  - PV uses ones-augmented V (65 cols/head): row 64 accumulates the softmax
    denominator; PE-transpose + reciprocal + one broadcast-stride multiply
    per head normalizes into cbig.
  - LayerNorm: accum-fused residual, Square/Sqrt on ScalarE, f16 output.
"""

import os
import sys

import numpy as np

try:
    import concourse.bass as bass  # noqa: F401
except ImportError:
    for _p in ("/opt/trn_rl_repo", "/root/.axon_site/_ro/trn_rl_repo"):
        if os.path.isdir(_p):
            sys.path.insert(0, _p)
            break
    import concourse.bass as bass  # noqa: F401

import ml_dtypes
import concourse.tile as tile
from concourse import bacc, bass_utils, mybir
from concourse.masks import make_identity

F32 = mybir.dt.float32
F16 = mybir.dt.float16
BF16 = mybir.dt.bfloat16
F8 = mybir.dt.float8e4
F8E5 = mybir.dt.float8e5
U8 = mybir.dt.uint8
AF = mybir.ActivationFunctionType
ALU = mybir.AluOpType
DR = mybir.MatmulPerfMode.DoubleRow
NP8 = mybir.dt.np(F8)
NPBF = ml_dtypes.bfloat16

B, N_Q, N_KV, D, H, HD = 2, 2048, 2048, 512, 8, 64
N_CORES = 8
QBLK = N_Q // 4          # 512 query rows per core
DC = D // 128            # 4 partition chunks of the model dim
CP = DC // 2             # 2 chunk-pairs (fp8 DoubleRow)
KC = N_KV // 128         # 16 key chunks
KCP = KC // 2            # 8 key chunk-pairs
HP = H // 2              # 4 head pairs
LN_EPS = 1e-5
SM_SCALE = 0.125
EXP_BIAS = -2.0          # probs = exp(s/8 - 2); e5m2 has no reachable cliff

# Schraudolph constants for e5m2 output via uint8: u8 = A8*s_raw + B8
A8 = 4.0 * np.log2(np.e) * SM_SCALE
B8 = 60.0 + 4.0 * np.log2(np.e) * EXP_BIAS - 4.0 * np.log2(1.0301)

# exp engine per (unit, kc): even kc -> ScalarE ACT exp, odd kc -> VectorE
# Schraudolph, so the two exps of a unit run concurrently on both engines.
# A few units send both halves to ACT to balance total throughput (36/28).
BOTH_ACT_UNITS = {3, 11, 19, 27}


def _build_program():
    nc = bacc.Bacc("TRN2", target_bir_lowering=False, debug=False,
                   num_devices=N_CORES)

    def din(name, shape, dt):
        return nc.dram_tensor(name, shape, dt, kind="ExternalInput").ap()

    # batched input tensors (few, large DMAs)
    ta = din("ta", [128, 4096], BF16)    # xqt c0..3 (2048) | wq c0..3 (2048)
    ta2 = din("ta2", [128, 2048], BF16)  # wk c0..3
    tb = [din(f"tb{h}", [128, 4096], BF16) for h in range(2)]  # kvT [c(4) x 1024keys]
    tc_ = din("tc", [128, 10240], F8)    # kvt8 DR (2x4096) | wv8 DR (2x1024)
    td = din("td", [128, 2048], F16)     # xqb residual (query + bv)
    te = din("te", [128, 1028], F32)     # bq (4) | gamma (512) | beta (512)
    out = nc.dram_tensor("out", [QBLK, D], F16, kind="ExternalOutput").ap()

    with tile.TileContext(nc) as tc:
        with (
            tc.tile_pool(name="persist", bufs=1) as pp,
            tc.tile_pool(name="work", bufs=2) as wkp,
            tc.tile_pool(name="small", bufs=8) as smp,
            tc.tile_pool(name="sc_ps", bufs=3, space="PSUM") as scps,
            tc.tile_pool(name="pv_ps", bufs=1, space="PSUM") as pvps,
        ):
            # ---- persistent tiles + loads ----
            a_sb = pp.tile([128, 4096], BF16, name="ta", tag="ta")
            a2_sb = pp.tile([128, 2048], BF16, name="ta2", tag="ta2")
            b_sb = [pp.tile([128, 4096], BF16, name=f"tb{h}", tag=f"tb{h}")
                    for h in range(2)]
            c_sb = pp.tile([128, 10240], F8, name="tc", tag="tc")
            d_sb = pp.tile([128, 2048], F16, name="td", tag="td")
            e_sb = pp.tile([128, 1028], F32, name="te", tag="te")

            # four DMA rings; W blocks are m-major and the fp8 tensor is
            # key-quarter-major so every proj step waits only on its slice.
            nc.sync.dma_start(e_sb[:, 0:4], te[:, 0:4])
            nc.sync.dma_start(a_sb[:, 0:3072], ta[:, 0:3072])     # xqt + wq m0,m1
            nc.sync.dma_start(a2_sb[:, 0:1024], ta2[:, 0:1024])   # wk m0,m1
            nc.sync.dma_start(b_sb[0][:], tb[0])
            nc.sync.dma_start(a_sb[:, 3072:4096], ta[:, 3072:4096])
            nc.gpsimd.dma_start(c_sb[:, 8192:10240], tc_[:, 8192:10240])  # wv
            nc.gpsimd.dma_start(c_sb[:, 0:2048], tc_[:, 0:2048])          # keys q0
            nc.gpsimd.dma_start(c_sb[:, 2048:4096], tc_[:, 2048:4096])    # keys q1
            nc.gpsimd.dma_start(c_sb[:, 4096:8192], tc_[:, 4096:8192])    # keys q2,q3
            nc.scalar.dma_start(d_sb[:], td)
            nc.scalar.dma_start(b_sb[1][:], tb[1])
            nc.scalar.dma_start(e_sb[:, 4:1028], te[:, 4:1028])
            nc.scalar.dma_start(a2_sb[:, 1024:2048], ta2[:, 1024:2048])

            def xqt_c(c):
                return a_sb[:, c * 512:(c + 1) * 512]

            def wq_cm(c, m):
                return a_sb[:, 2048 + m * 512 + c * 128: 2048 + m * 512 + (c + 1) * 128]

            def wk_cm(c, m):
                return a2_sb[:, m * 512 + c * 128: m * 512 + (c + 1) * 128]

            def kvt8_t(cp, t):
                # key-quarter kq = t//4 block: [kq][cp][i][512 keys]
                kq, tr = t // 4, t % 4
                base = kq * 2048 + cp * 1024
                return c_sb[:, base:base + 1024].rearrange(
                    "p (i n) -> p i n", i=2)[:, :, tr * 128:(tr + 1) * 128]

            def wv_v(cp):
                return c_sb[:, 8192 + cp * 1024: 8192 + (cp + 1) * 1024].rearrange(
                    "p (i n) -> p i n", i=2)

            xqb = d_sb[:, 0:2048]
            gamma = e_sb[:, 4:516]
            beta = e_sb[:, 516:1028]

            ident = pp.tile([128, 128], F32, name="ident", tag="ident")
            make_identity(nc, ident[:])
            ebias = pp.tile([128, 1], F32, name="ebias", tag="ebias")
            nc.vector.memset(ebias[:], float(EXP_BIAS))
            eps_sb = pp.tile([128, 1], F32, name="eps", tag="eps")
            nc.vector.memset(eps_sb[:], float(LN_EPS))

            qt_sb = [pp.tile([128, 1024], F8, name=f"qt{g}", tag=f"qt{g}") for g in range(2)]
            kt_sb = [pp.tile([128, 4096], F8, name=f"kt{g}", tag=f"kt{g}") for g in range(2)]
            vaug_sb = [pp.tile([128, 2048], F8, name=f"va{t}", tag=f"va{t}")
                       for t in range(KCP)]
            cbig = pp.tile([128, 2048], F32, name="cbig", tag="cbig")

            def proj_qt(m):
                ps = scps.tile([128, 512], F32, name="ps_qt", tag="sc")
                for c in range(DC):
                    nc.tensor.matmul(ps[:], wq_cm(c, m), xqt_c(c),
                                     start=(c == 0), stop=(c == DC - 1))
                nc.scalar.add(
                    qt_sb[m // 2][:, (m % 2) * 512:(m % 2 + 1) * 512],
                    ps[:], e_sb[:, m:m + 1])

            def proj_kt(m, nbb):
                # keys nbb*1024 .. +1024 of output-dim chunk m (bf16, no bias)
                ps = scps.tile([128, 1024], F32, name="ps_kt", tag="sc")
                for nn in range(2):
                    for c in range(DC):
                        mov = b_sb[nbb][:, c * 1024 + nn * 512: c * 1024 + (nn + 1) * 512]
                        nc.tensor.matmul(ps[:, nn * 512:(nn + 1) * 512],
                                         wk_cm(c, m), mov,
                                         start=(c == 0), stop=(c == DC - 1))
                dst = kt_sb[m // 2][:, (m % 2) * 2048 + nbb * 1024:(m % 2) * 2048 + (nbb + 1) * 1024]
                if m < 2:
                    nc.vector.tensor_copy(dst, ps[:])
                else:
                    nc.scalar.copy(dst, ps[:])

            def proj_v(t):
                ps = scps.tile([128, 512], F32, name="ps_v", tag="sc")
                for cp in range(CP):
                    nc.tensor.matmul(
                        ps[:], kvt8_t(cp, t), wv_v(cp),
                        start=(cp == 0), stop=(cp == CP - 1), perf_mode=DR)
                va4 = vaug_sb[t // 2][:].rearrange("p (h i c) -> p h i c", h=H, i=2)
                dst = va4[:, :, t % 2, 0:HD]
                src = ps[:].rearrange("p (h c) -> p h c", h=H)
                if t % 2 == 0:
                    nc.scalar.copy(dst, src)
                else:
                    nc.vector.tensor_copy(dst, src)

            def vaug_ones(kcp):
                # contiguous pre-fill 1.0; proj_v then overwrites cols 0..63 of
                # each 128-block, leaving col 64 = 1.0 (65..127 unused padding)
                nc.gpsimd.memset(vaug_sb[kcp][:], 1.0)

            # ---- attention ----
            # head h lives on partitions 64*(h%2)..+64 of chunk m=h//2 (no
            # permutation needed). Plain fp8 (no DR): stationary [64,128] gets
            # FWL-fast LDWEIGHTS; the two heads of a pair occupy disjoint row
            # halves -> both matmuls run concurrently, full-array utilization.
            def scores(hp, kc):
                g = hp // 2
                mb = hp % 2
                psc = scps.tile([128, 1024], F32, name="psc", tag="sc")
                for j in range(2):
                    b = 64 * j
                    nc.tensor.matmul(
                        psc[:, j * 512:(j + 1) * 512],
                        kt_sb[g][b:b + 64, mb * 2048 + kc * 128: mb * 2048 + (kc + 1) * 128],
                        qt_sb[g][b:b + 64, mb * 512:(mb + 1) * 512],
                        start=True, stop=True,
                        tile_position=(b, 0))
                return psc

            def exp_op(hp, kc, psc, pt, use_dve):
                parity = kc % 2
                pview = pt[:].rearrange("p (j i q) -> p j i q", j=2, i=2)[:, :, parity, :]
                sview = psc[:].rearrange("p (j q) -> p j q", j=2)
                if use_dve:
                    # uint8 convert saturates negatives to 0 -> prob +0.0
                    nc.vector.tensor_scalar(
                        out=pview.bitcast(U8), in0=sview, scalar1=float(A8),
                        scalar2=float(B8), op0=ALU.mult, op1=ALU.add)
                else:
                    nc.scalar.activation(pview, sview, AF.Exp,
                                         bias=ebias[:], scale=float(SM_SCALE))

            def pv(hp, kcp, ppv, pt):
                for j in range(2):
                    h = 2 * hp + j
                    va4 = vaug_sb[kcp][:].rearrange("p (h i c) -> p h i c", h=H, i=2)
                    nc.tensor.matmul(
                        ppv[j][:],
                        va4[:, h, :, 0:HD + 1],
                        pt[:].rearrange("p (j i q) -> p j i q", j=2, i=2)[:, j],
                        start=(kcp == 0), stop=(kcp == KCP - 1), perf_mode=DR,
                        skip_group_check=True)

            resid_sb, var_sb = {}, {}

            def finish(hp, then_ln=False):
                cts = []
                for j in range(2):
                    ct = wkp.tile([HD + 1, QBLK], F32, name=f"ct{j}", tag=f"ct{j}")
                    if j == 0:
                        nc.scalar.copy(ct[:], ppvs[hp][j][:])
                    else:
                        nc.vector.tensor_copy(ct[:], ppvs[hp][j][:])
                    cts.append(ct)
                for j in range(2):
                    h = 2 * hp + j
                    ptr = scps.tile([128, 4 * 66], F32, name="ptr", tag="sc")
                    for q in range(4):
                        nc.tensor.transpose(
                            ptr[:, q * 66:q * 66 + 65],
                            cts[j][:, q * 128:(q + 1) * 128],
                            ident[0:HD + 1, 0:HD + 1])
                    linv4 = smp.tile([128, 4], F32, name="linv", tag="linv")
                    nc.vector.reciprocal(
                        linv4[:], ptr[:].rearrange("p (q c) -> p q c", q=4)[:, :, HD])
                    # ctx[:, q, h*HD:+HD] = ptr[:, q, :HD] * linv[q]  (one op/head)
                    cview = cbig[:].rearrange("p (q d) -> p q d", q=4)[:, :, h * HD:(h + 1) * HD]
                    pview = ptr[:].rearrange("p (q c) -> p q c", q=4)[:, :, 0:HD]
                    lbc = linv4[:].unsqueeze(2).to_broadcast((128, 4, HD))
                    nc.vector.tensor_tensor(
                        out=cview, in0=pview, in1=lbc, op=ALU.mult)
                    if then_ln and j == 1:
                        for q in range(4):
                            ln_head(q)
                        ln_tail()

            def ln_head(q):
                resid = pp.tile([128, 512], F32, name=f"res{q}", tag=f"res{q}")
                rowsum = smp.tile([128, 1], F32, name="rs", tag=f"rs{q}")
                nc.vector.scalar_tensor_tensor(
                    out=resid[:], in0=cbig[:, q * 512:(q + 1) * 512], scalar=0.0,
                    in1=xqb[:, q * 512:(q + 1) * 512],
                    op0=ALU.bypass, op1=ALU.add, accum_out=rowsum[:])
                sq = wkp.tile([128, 512], F32, name="sq", tag="sq")
                sqs = smp.tile([128, 1], F32, name="sqs", tag=f"sqs{q}")
                nc.scalar.activation(sq[:], resid[:], AF.Square, accum_out=sqs[:])
                mu = smp.tile([128, 1], F32, name="mu", tag=f"mu{q}")
                nc.vector.tensor_scalar_mul(mu[:], rowsum[:], 1.0 / D)
                musq = smp.tile([128, 1], F32, name="musq", tag=f"musq{q}")
                nc.vector.tensor_tensor(out=musq[:], in0=mu[:], in1=mu[:], op=ALU.mult)
                var = smp.tile([128, 1], F32, name="var", tag=f"var{q}")
                nc.vector.scalar_tensor_tensor(
                    out=var[:], in0=sqs[:], scalar=1.0 / D, in1=musq[:],
                    op0=ALU.mult, op1=ALU.subtract)
                var_sb[q] = var
                # tA = (resid - mu) * gamma  (gamma broadcast tile)
                tA = pp.tile([128, 512], F32, name=f"tA{q}", tag=f"tA{q}")
                nc.vector.scalar_tensor_tensor(
                    out=tA[:], in0=resid[:], scalar=mu[:], in1=gamma,
                    op0=ALU.subtract, op1=ALU.mult)
                resid_sb[q] = tA

            def ln_tail():
                for q in range(4):
                    std = smp.tile([128, 1], F32, name="std", tag=f"std{q}")
                    nc.scalar.activation(std[:], var_sb[q][:], AF.Sqrt, bias=eps_sb[:])
                    inv = smp.tile([128, 1], F32, name="inv", tag=f"inv{q}")
                    nc.vector.reciprocal(inv[:], std[:])
                    o = wkp.tile([128, 512], F16, name="o", tag="o")
                    nc.vector.scalar_tensor_tensor(
                        out=o[:], in0=resid_sb[q][:], scalar=inv[:], in1=beta,
                        op0=ALU.mult, op1=ALU.add)
                    nc.sync.dma_start(out[q * 128:(q + 1) * 128, :], o[:])

            # ---- schedule ----
            for m in range(4):
                proj_qt(m)
            proj_kt(0, 0)
            proj_kt(1, 0)
            for t in range(4):
                if t % 2 == 0:
                    vaug_ones(t // 2)
                proj_v(t)
            proj_kt(0, 1)
            proj_kt(1, 1)
            for t in range(4, 8):
                if t % 2 == 0:
                    vaug_ones(t // 2)
                proj_v(t)

            # v t=8..15 and kt m=2,3 interleave across hp=0's 16 kc slots
            late_work = [("v", 8), ("v", 9), ("v", 10), ("v", 11),
                         ("v", 12), ("v", 13), ("v", 14), ("v", 15),
                         ("kt", 2, 0), ("kt", 3, 0), ("kt", 2, 1), ("kt", 3, 1)]
            ppvs = {}
            pending = None
            for hp in range(HP):
                ppv = [pvps.tile([HD + 1, QBLK], F32, name=f"pv{j}", tag=f"pv{j}")
                       for j in range(2)]
                ppvs[hp] = ppv
                for kc in range(KC):
                    if hp == 0 and late_work and kc % 2 == 0 or hp == 0 and kc >= 8 and late_work:
                        w = late_work.pop(0)
                        if w[0] == "kt":
                            proj_kt(w[1], w[2])
                        else:
                            t = w[1]
                            if t % 2 == 0:
                                vaug_ones(t // 2)
                            proj_v(t)
                    psc = scores(hp, kc)
                    kcp = kc // 2
                    unit = hp * 8 + kcp
                    if kc % 2 == 0:
                        tag = "pta" if unit % 2 == 0 else "ptb"
                        pt_cur = wkp.tile([128, 2048], F8E5, name=tag, tag=tag)
                    use_dve = (kc % 2 == 1) and unit not in BOTH_ACT_UNITS
                    exp_op(hp, kc, psc, pt_cur, use_dve)
                    if kc % 2 == 1:
                        pv(hp, kcp, ppv, pt_cur)
                if pending is not None:
                    finish(pending)
                pending = hp
            finish(pending, then_ln=True)

    nc.compile()
    return nc


_PROGRAM = None


def _get_program():
    global _PROGRAM
    if _PROGRAM is None:
        _PROGRAM = _build_program()
    return _PROGRAM


def _dr_pack(m):
    """[512, C] contraction-major -> per chunk-pair [128, 2*C] fp8 tiles."""
    res = []
    for cp in range(CP):
        t = m.reshape(2, 2, 128, m.shape[1])[cp].transpose(1, 0, 2).reshape(128, -1)
        res.append(np.ascontiguousarray(t, dtype=NP8))
    return res


def _chunks_bf(m):
    """[512, C] -> [128, 4*C] bf16: 4 row-chunks side by side."""
    return np.ascontiguousarray(
        m.reshape(4, 128, m.shape[1]).transpose(1, 0, 2).reshape(128, -1), NPBF)


def _wblocks_mm(w):
    """[512, 512] -> [128, 2048] bf16, m-major: [p, m*512 + c*128 + r] =
    w[c*128 + p, m*128 + r]."""
    return np.ascontiguousarray(
        w.reshape(4, 128, 4, 128).transpose(1, 2, 0, 3).reshape(128, 2048), NPBF)


def _make_in_maps(query, key_value, Wq, bq, Wk, bk, Wv, bv, ln_gamma, ln_beta):
    f16, f32 = np.float16, np.float32
    ta_w = _wblocks_mm(Wq)                       # [128, 2048] m-major
    ta2 = _wblocks_mm(Wk)                        # [128, 2048] m-major
    wv8 = _dr_pack(Wv)
    te = np.empty((128, 1028), f32)
    te[:, 0:4] = bq.reshape(4, 128).T
    te[:, 4:516] = ln_gamma[None, :]
    te[:, 516:1028] = ln_beta[None, :]

    tbs, tcs = [], []
    for b in range(B):
        kvT = np.ascontiguousarray(key_value[b].T)          # [512, 2048]
        kb = _chunks_bf(kvT)                                # [128, 4*2048]
        kb4 = kb.reshape(128, 4, 2048)
        tb0 = np.ascontiguousarray(kb4[:, :, 0:1024].reshape(128, 4096))
        tb1 = np.ascontiguousarray(kb4[:, :, 1024:2048].reshape(128, 4096))
        kvt8 = _dr_pack(kvT)
        tc_ = np.empty((128, 10240), NP8)
        for kq in range(4):
            for cp in range(CP):
                # [kq][cp][i(2) x 512 keys]
                blk = kvt8[cp].reshape(128, 2, 2048)[:, :, kq * 512:(kq + 1) * 512]
                tc_[:, kq * 2048 + cp * 1024:kq * 2048 + (cp + 1) * 1024] = \
                    blk.reshape(128, 1024)
        tc_[:, 8192:9216] = wv8[0]
        tc_[:, 9216:10240] = wv8[1]
        tbs.append((tb0, tb1))
        tcs.append(tc_)

    in_maps = []
    for core in range(N_CORES):
        b, qb = divmod(core, 4)
        blk = query[b, qb * QBLK:(qb + 1) * QBLK, :]
        ta = np.empty((128, 4096), NPBF)
        ta[:, 0:2048] = _chunks_bf(np.ascontiguousarray(blk.T))
        ta[:, 2048:4096] = ta_w
        xqb = blk + bv[None, :]
        td = np.ascontiguousarray(
            xqb.reshape(4, 128, 512).transpose(1, 0, 2).reshape(128, 2048), f16)
        m = {"ta": ta, "ta2": ta2, "tb0": tbs[b][0], "tb1": tbs[b][1],
             "tc": tcs[b], "td": td, "te": te}
        in_maps.append(m)
    return in_maps


def kernel(query, key_value, Wq, bq, Wk, bk, Wv, bv, ln_gamma, ln_beta,
           _trace=False, _trace_kwargs=None):
    args = [np.asarray(a, dtype=np.float32) for a in
            (query, key_value, Wq, bq, Wk, bk, Wv, bv, ln_gamma, ln_beta)]
    nc = _get_program()
    in_maps = _make_in_maps(*args)
    res = bass_utils.run_bass_kernel_spmd(
        nc, in_maps, core_ids=list(range(N_CORES)), trace=_trace,
        **(_trace_kwargs or {}))
    out = np.empty((B, N_Q, D), np.float32)
    for core in range(N_CORES):
        b, qb = divmod(core, 4)
        out[b, qb * QBLK:(qb + 1) * QBLK, :] = res.results[core]["out"].astype(np.float32)
    if _trace:
        return out, res
    return out
